# revision 15
# baseline (speedup 1.0000x reference)
"""Trainium2 Bass kernel for nn_EventPairCompositionModel (v2).

Strategy (data-parallel over batch, 8 cores, B=512 -> 64 per core):
  - The context MLP over all 8192 (b,n) pairs per core is 99.2% of the
    FLOPs and stays on device.  The host packs each pair's concatenated
    4x300-dim fp8 embedding directly into the PE DoubleRow operand layout
    (K-pairs on partitions), so the device streams one big contiguous
    HWDGE DMA per 512-pair group instead of 65 SWDGE row gathers (which
    capped the old kernel at ~25GB/s/queue for 88us).
  - W1 is host-permuted to the identical K-pair order; the 1200-dim
    contraction runs as 5 DoubleRow passes x 4 M-tiles per group.
  - The tiny event path (64 rows, <1% of FLOPs) plus distance features
    are computed on host in f32; the event representations enter the
    device as the fp8 stationary operand of the cosine-numerator matmul,
    exactly as the device would have quantized them.
  - Cosine numerator (pT) and |c|^2 (pN via ones-matmul on squared
    activations) rows go PSUM -> [1,512] SBUF rows -> direct SBUF->SBUF
    DMA respread into [128, 64] tail tiles (partition = 2b + n/64, one
    hop, no DRAM roundtrip), giving the KNRM tail full 128-partition
    parallelism; the first three tail quarters run overlapped inside
    the main loop and the final quarter splits its square/exp work
    across Scalar and Vector to shorten the exposed serial chain.
  - All activations (relu / square / exp / ln) live in the single
    natural_log_exp activation table -> one table load, no swaps.
All 8 cores run the identical program on their own batch shard (SPMD,
no collectives); host concatenates the 8 (64,1) outputs.
"""

import numpy as np
import ml_dtypes

import concourse.bacc as bacc
import concourse.tile as tile
import concourse.mybir as mybir
from concourse.bass_utils import run_bass_kernel_spmd

F32 = mybir.dt.float32
F8 = mybir.dt.float8e4
AF = mybir.ActivationFunctionType
DR = mybir.MatmulPerfMode.DoubleRow
MUL = mybir.AluOpType.mult

# Problem shapes (hardcoded per spec)
B, N, C, E = 512, 128, 4, 300
V = 50000
H1, H2 = 512, 256
NF, NK = 8, 11
NCORES = 8
BC = B // NCORES            # 64 batches per core
GROUPS = 16                 # groups of 512 (b,n) pairs
BN = BC * N                 # 8192 pairs per core
NPAIR = 600                 # K-pairs of the 1200-dim concat input
KP5 = 5                     # DoubleRow K-passes (128 pairs each, padded)
XS = 16.0                   # fp8 table scale
HS = 8.0                    # fp8 hidden-activation scale

MUS = [1.0, 0.9, 0.7, 0.5, 0.3, 0.1, -0.1, -0.3, -0.5, -0.7, -0.9]
SIGMAS = [1e-3] + [0.1] * 10

_PROGRAM_CACHE = {}


def _build_v2():
    if "v2" in _PROGRAM_CACHE:
        return _PROGRAM_CACHE["v2"]

    nc = bacc.Bacc("TRN2", target_bir_lowering=False, debug=False)

    # ---- DRAM I/O ----
    xop = nc.dram_tensor("xop", (GROUPS * 128, KP5 * 2 * 512), F8,
                         kind="ExternalInput")
    # first group's + W1's halves as separate contiguous tensors: strided
    # column-slice DMAs run at ~1/3 the contiguous rate
    xop0a = nc.dram_tensor("xop0a", (128, 2048), F8, kind="ExternalInput")
    xop0b = nc.dram_tensor("xop0b", (128, 3072), F8, kind="ExternalInput")
    w1pa = nc.dram_tensor("w1pa", (128, 2048), F8, kind="ExternalInput")
    w1pb = nc.dram_tensor("w1pb", (128, 3072), F8, kind="ExternalInput")
    w2q = nc.dram_tensor("w2q", (128, 4 * H2), F8, kind="ExternalInput")
    eh2q = nc.dram_tensor("eh2q", (128, 2 * 128), F8, kind="ExternalInput")
    # packed consts: 0-3 8*b1 | 4-5 8*b2 | 6 ne2 | 7-17 kpb | 18-28 sck
    # | 29 fs64 (rows 0-63) | 30 wkp (rows 0-31)
    ccd = nc.dram_tensor("ccd", (128, 31), F32, kind="ExternalInput")
    out_d = nc.dram_tensor("out", (BC, 1), F32, kind="ExternalOutput")

    with tile.TileContext(nc) as tc:
        with (
            tc.tile_pool(name="consts", bufs=1) as cpool,
            tc.tile_pool(name="xt", bufs=6) as xtpool,
            tc.tile_pool(name="s1", bufs=4) as s1pool,
            tc.tile_pool(name="s2", bufs=4) as s2pool,
            tc.tile_pool(name="csq", bufs=3) as csqpool,
            tc.tile_pool(name="rows", bufs=4) as rowpool,
            tc.tile_pool(name="tail", bufs=1) as tpool,
            tc.tile_pool(name="pm1", bufs=2, space="PSUM") as pm1,
            tc.tile_pool(name="pm2", bufs=2, space="PSUM") as pm2,
            tc.tile_pool(name="ptn", bufs=1, space="PSUM") as ptn,
            tc.tile_pool(name="pnn", bufs=1, space="PSUM") as pnn,
            tc.tile_pool(name="dsc", bufs=1, space="DRAM") as dpool,
        ):
            # ---- weights / consts (scalar HWDGE queue); w1p split into
            # k-slices so the first MLP1 chain can start ASAP ----
            w1p_s = cpool.tile([128, KP5 * 2 * H1], F8)
            nc.scalar.dma_start(w1p_s[:, 0:2048], w1pa.ap())
            nc.scalar.dma_start(w1p_s[:, 2048:5120], w1pb.ap())
            cc_s = cpool.tile([128, 31], F32)
            nc.gpsimd.dma_start(cc_s[:], ccd.ap())
            eh2q_s = cpool.tile([128, 2 * 128], F8)
            nc.gpsimd.dma_start(eh2q_s[:], eh2q.ap())
            w2q_s = cpool.tile([128, 4 * H2], F8)
            nc.scalar.dma_start(w2q_s[:], w2q.ap())
            b1_s = cc_s[:, 0:4]
            b2_s = cc_s[:, 4:6]
            ne2_s = cc_s[:, 6:7]
            kpb_s = cc_s[:, 7:18]
            fs_s = cc_s[0:BC, 29:30]
            wkp_s = cc_s[0:32, 30:31]
            ones8 = cpool.tile([128, 2 * 16], F8)
            nc.vector.memset(ones8[:], 1.0)
            eps_s = cpool.tile([128, 1], F32)
            nc.vector.memset(eps_s[:], 1e-20)
            # KNRM per-kernel scale/bias expanded along the free dim so the
            # 11 per-k square ops become 3 broadcast vector ops per quarter
            kcs_s = cpool.tile([128, NK * 64], F32)
            nc.vector.tensor_copy(
                kcs_s[:].rearrange("p (k j) -> p k j", k=NK),
                cc_s[:, 18:29].rearrange("p (k o) -> p k o", k=NK)
                .broadcast_to([128, NK, 64]),
            )
            kcb_s = cpool.tile([128, NK * 64], F32)
            nc.vector.tensor_copy(
                kcb_s[:].rearrange("p (k j) -> p k j", k=NK),
                cc_s[:, 7:18].rearrange("p (k o) -> p k o", k=NK)
                .broadcast_to([128, NK, 64]),
            )

            # ---- xop group tiles: prefetch on both HWDGE queues ----
            xts = {}

            def fetch(g):
                xt = xtpool.tile([128, KP5 * 2 * 512], F8, tag="xt",
                                 name=f"xt_{g}")
                # all xop tiles ride the sync HWDGE queue; the scalar queue
                # carries only the weights/consts at startup.
                eng = nc.scalar if g in (1, 2) else nc.sync
                if g == 0:
                    eng.dma_start(xt[:, 0:2048], xop0a.ap())
                    eng.dma_start(xt[:, 2048:5120], xop0b.ap())
                else:
                    eng.dma_start(xt[:], xop.ap()[128 * g:128 * (g + 1), :])
                xts[g] = xt

            for g in range(4):
                fetch(g)

            # DRAM scratch rows; flat index b*128 + n reads back as
            # [64, 64] per half with partition = 2b + n//64.
            trd = dpool.tile([1, BN], F32, name="trd")
            ncd = dpool.tile([1, BN], F32, name="ncd")

            def w1v(k5, m):
                return w1p_s[:].rearrange(
                    "p (k q m) -> p k q m", k=KP5, q=2
                )[:, k5, :, 128 * m:128 * (m + 1)]

            def xv(xt, k5):
                return xt[:].rearrange(
                    "p (k q i) -> p k q i", k=KP5, q=2
                )[:, k5, :, :]

            # per-group state kept for the pipelined deferred emission
            state = {}
            state2 = {}

            def emit_mlp1(g):
                # b1 is folded into the matmul (constant operand pair-row),
                # so relu1 needs no bias and fuses into one two-bank ACT
                # per m-pair -> Scalar stays well ahead of the PE.
                xt = xts[g]
                s1 = s1pool.tile([128, 4 * 512], F8, tag="s1", name=f"s1_{g}")
                for half in range(2):
                    pp = pm1.tile([128, 1024], F32, tag="pm1",
                                  name=f"p1_{g}_{half}")
                    for mi in range(2):
                        m = 2 * half + mi
                        for k5 in range(KP5):
                            nc.tensor.matmul(
                                pp[:, 512 * mi:512 * (mi + 1)],
                                w1v(k5, m), xv(xt, k5),
                                start=(k5 == 0), stop=(k5 == KP5 - 1),
                                perf_mode=DR,
                            )
                    nc.scalar.activation(
                        s1[:, 1024 * half:1024 * (half + 1)], pp[:], AF.Relu,
                        scale=0.5,
                    )
                state[g] = s1

            def emit_mlp2(g, mid=None, last=False):
                s1 = state.pop(g)
                s28 = s2pool.tile([128, 2 * 512], F8, tag="s28", name=f"s28_{g}")
                p2 = [
                    pm2.tile([128, 512], F32, tag="pm2", name=f"p2_{g}_{m}")
                    for m in range(2)
                ]
                for j in range(2):
                    for m in range(2):
                        nc.tensor.matmul(
                            p2[m][:],
                            w2q_s[:].rearrange("p (u m) -> p u m", u=4)[
                                :, 2 * j:2 * j + 2, 128 * m:128 * (m + 1)
                            ],
                            s1[:].rearrange("p (u i) -> p u i", u=4)[
                                :, 2 * j:2 * j + 2, :
                            ],
                            start=(j == 0), stop=(j == 1), perf_mode=DR,
                        )
                    if j == 0 and mid is not None:
                        mid()
                for m in range(2):
                    nc.scalar.activation(
                        s28[:, 512 * m:512 * (m + 1)], p2[m][:], AF.Relu,
                        bias=b2_s[:, m:m + 1], scale=1.0,
                    )
                csq8 = csqpool.tile([128, 2 * 512], F8, tag="csq",
                                    name=f"csq_{g}")
                if last:
                    nc.vector.tensor_mul(csq8[:, 0:512], s28[:, 0:512],
                                         s28[:, 0:512])
                    nc.scalar.activation(csq8[:, 512:1024], s28[:, 512:1024],
                                         AF.Square)
                else:
                    nc.vector.tensor_mul(csq8[:], s28[:], s28[:])
                state2[g] = (s28, csq8)

            def emit_tnmm(g, last=False):
                s28, csq8 = state2.pop(g)
                s28v = s28[:].rearrange("p (u i) -> p u i", u=2)
                pt = ptn.tile([16, 512], F32, tag="ptn", name=f"ptn_{g}")
                pn = pnn.tile([16, 512], F32, tag="pnn", name=f"pnn_{g}")
                for z in range(4):
                    b = 4 * g + z
                    nc.tensor.matmul(
                        pt[0:16, 128 * z:128 * (z + 1)],
                        eh2q_s[:].rearrange("p (u i) -> p u i", u=2)[
                            :, :, b:b + 16
                        ],
                        s28v[:, :, 128 * z:128 * (z + 1)],
                        start=True, stop=True, perf_mode=DR,
                    )
                nc.tensor.matmul(
                    pn[:],
                    ones8[:].rearrange("p (q m) -> p q m", q=2),
                    csq8[:].rearrange("p (u i) -> p u i", u=2),
                    start=True, stop=True, perf_mode=DR,
                )
                trow = rowpool.tile([1, 512], F32, tag="trow", name=f"tr_{g}")
                nc.vector.tensor_copy(trow[:], pt[0:1, :])
                nrow = rowpool.tile([1, 512], F32, tag="nrow", name=f"nr_{g}")
                (nc.scalar.copy if last else nc.vector.tensor_copy)(
                    nrow[:], pn[0:1, :])
                nc.sync.dma_start(
                    trw[8 * g:8 * (g + 1), :],
                    trow[:].rearrange("o (z j) -> o z j", z=8))
                nc.sync.dma_start(
                    ncq[8 * g:8 * (g + 1), :],
                    nrow[:].rearrange("o (z j) -> o z j", z=8))

            # ---- tail tiles ([128, *]; half h uses partitions 64h..) ----
            trw = tpool.tile([128, 64], F32)
            ncq = tpool.tile([128, 64], F32)
            prodn = tpool.tile([128, 64], F32)
            lnp = tpool.tile([128, 64], F32)
            nrmf = tpool.tile([128, 64], F32)
            trans = tpool.tile([128, 64], F32)
            yk = tpool.tile([128, NK * 64], F32)
            ekb = tpool.tile([128, NK * 64], F32)
            pool32 = tpool.tile([128, 32], F32)
            tpl = tpool.tile([32, 128], F32)
            tps = tpool.tile([32, 64], F32)
            kpc = tpool.tile([32, 64], F32)
            kpl = tpool.tile([32, 64], F32)
            kpw = tpool.tile([32, 64], F32)
            kplb = tpool.tile([BC, 32], F32)
            kps = tpool.tile([BC, 1], F32)
            tot = tpool.tile([BC, 1], F32)
            emx = tpool.tile([BC, 1], F32)
            emx1 = tpool.tile([BC, 1], F32)
            outs = tpool.tile([BC, 1], F32)
            nc.vector.memset(pool32[:], 0.0)

            def tail_dma(g0, ng, eng=None):
                eng = eng or nc.sync
                np_ = 8 * ng                          # partitions 8g0..
                ps = slice(8 * g0, 8 * g0 + np_)
                fq = slice(512 * g0, 512 * (g0 + ng))
                eng.dma_start(
                    trw[ps, :],
                    trd[:, fq].rearrange("o (p j) -> (o p) j", p=np_),
                )
                eng.dma_start(
                    ncq[ps, :],
                    ncd[:, fq].rearrange("o (p j) -> (o p) j", p=np_),
                )

            def tail_dma15(eng):
                eng.dma_start(
                    trw[120:128, :],
                    trd[:, 7680:8192].rearrange("o (p j) -> (o p) j", p=8),
                )
                eng.dma_start(
                    ncq[120:128, :],
                    ncd[:, 7680:8192].rearrange("o (p j) -> (o p) j", p=8),
                )

            def tail_pre(g0, ng):
                """norm + trans for groups [g0, g0+ng) (32-aligned)."""
                np_ = 8 * ng
                ps = slice(8 * g0, 8 * g0 + np_)
                nc.vector.tensor_tensor(
                    out=prodn[ps, :], in0=ncq[ps, :],
                    in1=ne2_s[ps, :].broadcast_to([np_, 64]), op=MUL,
                )
                nc.scalar.activation(lnp[ps, :], prodn[ps, :], AF.Ln,
                                     bias=eps_s[ps, :])
                nc.scalar.activation(nrmf[ps, :], lnp[ps, :], AF.Exp,
                                     scale=-0.5)
                nc.vector.tensor_mul(trans[ps, :], trw[ps, :], nrmf[ps, :])

            def tail_yk(g0, ng, k0, k1):
                """squares for kernels [k0,k1) via broadcast TT (vector)."""
                np_ = 8 * ng
                ps = slice(8 * g0, 8 * g0 + np_)
                ks = slice(64 * k0, 64 * k1)
                nk = k1 - k0
                tb = trans[ps, :].rearrange(
                    "p (o j) -> p o j", o=1).broadcast_to([np_, nk, 64])
                nc.vector.tensor_tensor(
                    out=yk[ps, ks].rearrange("p (k j) -> p k j", k=nk),
                    in0=tb, in1=kcs_s[ps, ks].rearrange(
                        "p (k j) -> p k j", k=nk), op=MUL,
                )
                nc.vector.tensor_add(yk[ps, ks], yk[ps, ks], kcb_s[ps, ks])
                nc.vector.tensor_mul(yk[ps, ks], yk[ps, ks], yk[ps, ks])

            def tail_yk_sc(g0, ng, k0, k1):
                """squares for kernels [k0,k1) via Square ACTs (scalar)."""
                np_ = 8 * ng
                ps = slice(8 * g0, 8 * g0 + np_)
                for k in range(k0, k1):
                    sck = 1.0 / (SIGMAS[k] * 2.0 ** 0.5)
                    nc.scalar.activation(
                        yk[ps, 64 * k:64 * (k + 1)], trans[ps, :], AF.Square,
                        bias=kpb_s[ps, k:k + 1], scale=sck,
                    )

            def tail_pool(g0, ng, k0, k1):
                """exp + n-pooling for kernels [k0,k1)."""
                np_ = 8 * ng
                ps = slice(8 * g0, 8 * g0 + np_)
                ks = slice(64 * k0, 64 * k1)
                nc.scalar.activation(ekb[ps, ks], yk[ps, ks], AF.Exp,
                                     scale=-1.0)
                nc.vector.reduce_sum(
                    out=pool32[ps, k0:k1],
                    in_=ekb[ps, ks].rearrange("p (k n) -> p k n", k=k1 - k0),
                    axis=mybir.AxisListType.X,
                )

            def tail_c(q):
                """per-32-partition-block: transpose, pair-sum, ln, weight."""
                nc.vector.transpose(tpl[0:32, 32 * q:32 * (q + 1)],
                                    pool32[32 * q:32 * (q + 1), :])
                cs = slice(16 * q, 16 * (q + 1))
                tv = tpl[:, 32 * q:32 * (q + 1)].rearrange(
                    "p (b v) -> p b v", v=2
                )
                nc.vector.tensor_tensor(
                    out=tps[:, cs], in0=tv[:, :, 0], in1=tv[:, :, 1],
                    op=mybir.AluOpType.add,
                )
                nc.vector.tensor_scalar_max(kpc[:, cs], tps[:, cs], 1e-10)
                nc.scalar.activation(kpl[:, cs], kpc[:, cs], AF.Ln)
                nc.vector.tensor_tensor(
                    out=kpw[:, cs], in0=kpl[:, cs],
                    in1=wkp_s[:].broadcast_to([32, 16]), op=MUL,
                )
                if q % 2 == 1:
                    h = q // 2
                    nc.vector.transpose(
                        kplb[32 * h:32 * (h + 1), :],
                        kpw[:, 32 * h:32 * (h + 1)],
                    )

            # ---- main loop, software-pipelined: pT/pN of g-1 run between
            # MLP2(g)'s two K-passes so no matmul ever waits on a relu;
            # tail chunks trickle through in small emission bursts ----
            for g in range(GROUPS):
                if g + 4 < GROUPS:
                    fetch(g + 4)
                emit_mlp1(g)
                mid = (lambda gg=g: emit_tnmm(gg - 1)) if g > 0 else None
                emit_mlp2(g, mid=mid, last=(g == GROUPS - 1))
                if g in (5, 9, 13):
                    tail_pre(g - 5, 4)
                    tail_yk(g - 5, 4, 0, NK)
                elif g in (6, 10, 14):
                    tail_pool(g - 6, 4, 0, NK)
                elif g in (7, 11):
                    tail_c((g - 7) // 4)
            emit_tnmm(GROUPS - 1, last=True)
            tail_c(2)
            # final quarter: split the kernel range across Scalar and Vector
            # to shorten the exposed serial chain
            tail_pre(12, 4)
            tail_yk_sc(12, 4, 6, NK)
            tail_yk(12, 4, 0, 6)
            tail_pool(12, 4, 6, NK)
            tail_pool(12, 4, 0, 6)
            tail_c(3)

            nc.vector.reduce_sum(out=kps[:], in_=kplb[:],
                                 axis=mybir.AxisListType.X)
            nc.vector.tensor_add(tot[:], kps[:], fs_s[:])
            nc.scalar.activation(emx[:], tot[:], AF.Exp, scale=-1.0)
            nc.vector.tensor_scalar_add(emx1[:], emx[:], 1.0)
            nc.vector.reciprocal(outs[:], emx1[:])
            nc.sync.dma_start(out_d.ap(), outs[:])

    # Compile with only the combined ln+exp activation table visible so the
    # table-load pass emits a single load instead of ping-ponging between
    # the exp-only and ln-only tables (1.3us per swap on the Scalar engine).
    import concourse.bacc as bacc_mod
    orig_tables = bacc_mod.get_activation_tables

    def only_combined(arch):
        t = orig_tables(arch)
        keep = "natural_log_exp_and_others"
        return {k: (v if k == keep else set()) for k, v in t.items()}

    bacc_mod.get_activation_tables = only_combined
    try:
        nc.compile()
    finally:
        bacc_mod.get_activation_tables = orig_tables
    _PROGRAM_CACHE["v2"] = nc
    return nc


def _prep_shared(inputs):
    """Core-independent host prep: fp8 tables + permuted weights."""
    f8 = ml_dtypes.float8_e4m3fn
    table = np.asarray(inputs["event_table"], np.float32)     # (V+1, E)
    W1 = np.asarray(inputs["W1"], np.float32)                 # (H1, C*E)
    W2 = np.asarray(inputs["W2"], np.float32)                 # (H2, H1)
    b1 = np.asarray(inputs["b1"], np.float32)
    b2 = np.asarray(inputs["b2"], np.float32)

    tq8 = (table * XS).astype(f8)                             # (V+1, 300)
    tq_pairs = tq8.reshape(V + 1, NPAIR // C, 2)

    # K-pair t -> source element pair of the 1200-dim concat (pad -> -1)
    t = np.arange(128 * KP5)
    c = t // (NPAIR // C)
    r = t % (NPAIR // C)
    e0 = E * c + 2 * r
    elem = np.stack([e0, e0 + 1], axis=1)                     # (640, 2)
    elem[t >= NPAIR] = C * E                                  # pad col
    W1q = W1.astype(f8).astype(np.float32)
    W1pad = np.concatenate([W1q, np.zeros((H1, 1), np.float32)], axis=1)
    w1p = W1pad[:, elem]                                      # (512, 640, 2)
    w1p = w1p.reshape(H1, KP5, 128, 2).transpose(2, 1, 3, 0)  # (128,5,2,512)
    w1p = np.ascontiguousarray(w1p)
    w1p[88, 4, 0, :] = 16.0 * b1   # bias row: pairs with xop's const-1 row

    W2q = W2.astype(f8).astype(np.float32)
    w2q = W2q.T.reshape(4, 128, H2).transpose(1, 0, 2)        # (128, 4, 256)

    mus = np.array(MUS, np.float32)
    sig = np.array(SIGMAS, np.float32)
    sck = 1.0 / (sig * np.sqrt(2.0))
    cc = np.zeros((128, 31), np.float32)
    cc[:, 0:4] = 8.0 * b1.reshape(4, 128).T
    cc[:, 4:6] = 8.0 * b2.reshape(2, 128).T
    cc[:, 7:18] = np.tile((-mus * sck)[None, :], (128, 1))
    cc[:, 18:29] = np.tile(sck[None, :], (128, 1))

    return {
        "tq_pairs": tq_pairs,
        "consts": {
            "w1pa": np.ascontiguousarray(
                w1p.reshape(128, KP5 * 2 * H1)[:, 0:2048]).astype(f8),
            "w1pb": np.ascontiguousarray(
                w1p.reshape(128, KP5 * 2 * H1)[:, 2048:5120]).astype(f8),
            "w2q": np.ascontiguousarray(w2q.reshape(128, 4 * H2)).astype(f8),
        },
        "cc_base": cc,
    }


def _prep_core(inputs, shared, core):
    """Per-core host prep: operand packing + f32 event path."""
    f8 = ml_dtypes.float8_e4m3fn
    table = np.asarray(inputs["event_table"], np.float32)
    W1 = np.asarray(inputs["W1"], np.float32)
    b1 = np.asarray(inputs["b1"], np.float32)
    W2 = np.asarray(inputs["W2"], np.float32)
    b2 = np.asarray(inputs["b2"], np.float32)
    Wv = np.asarray(inputs["Wv"], np.float32)
    bv = np.asarray(inputs["bv"], np.float32)
    Wc = np.asarray(inputs["Wc"], np.float32)
    bc = np.asarray(inputs["bc"], np.float32)

    sl = slice(core * BC, (core + 1) * BC)
    ev = np.asarray(inputs["batch_event"][sl], np.int64)
    feats = np.asarray(inputs["batch_features"][sl], np.float32)
    dists = np.asarray(inputs["batch_distances"][sl], np.float32)
    ctx = np.asarray(inputs["batch_context"][sl], np.int64)

    # context operand: (8192, 4, 150, 2) fp8 -> [16*128, 5*2*512]
    g = shared["tq_pairs"][ctx.reshape(BN, C)]                # (8192,4,150,2)
    g = g.reshape(BN, NPAIR, 2)
    pad = np.zeros((BN, 128 * KP5 - NPAIR, 2), f8)
    g = np.concatenate([g, pad], axis=1)                      # (8192, 640, 2)
    g[:, NPAIR, 0] = 1.0           # const-1 row pairing with w1p's bias row
    xopa = g.reshape(GROUPS, 512, KP5, 128, 2).transpose(0, 3, 2, 4, 1)
    xop = np.ascontiguousarray(xopa.reshape(GROUPS * 128, KP5 * 2 * 512))

    # event path in f32 (exact reference math)
    ee = table[ev]                                            # (64, 4, 300)
    x = ee.reshape(BC, C * E)
    h1 = np.maximum(x @ W1.T + b1, 0.0)
    er = np.maximum(h1 @ W2.T + b2, 0.0)                      # (64, 256)
    eq = (HS * er).astype(f8).astype(np.float32)              # device quant
    ne2 = (eq * eq).sum(axis=1)                               # 64*|e|^2

    # eh2q layout [p, u, col]: value eq[col, 128u + p]
    eh = np.zeros((128, 2, 128), np.float32)
    eh[:, :, 0:BC] = eq.T.reshape(2, 128, BC).transpose(1, 0, 2)
    eh2q = np.ascontiguousarray(eh.reshape(128, 256)).astype(f8)

    var = np.log1p(np.exp(ee[:, 1, :] @ Wv.T + bv))           # (64, 9)
    dist_emb = np.exp(-(dists * dists) / var)
    extracted = np.concatenate([dist_emb, feats], axis=1)     # (64, 17)
    fs = extracted @ Wc[0, 0:NF + 9] + bc[0]                  # (64,)

    cc = shared["cc_base"].copy()
    cc[:, 6] = np.repeat(ne2, 2)
    cc[0:BC, 29] = fs
    cc[0:NK, 30] = 0.01 * Wc[0, NF + 9:]

    m = dict(shared["consts"])
    m.update({
        "xop": xop, "eh2q": eh2q, "ccd": cc,
        "xop0a": np.ascontiguousarray(xop[0:128, 0:2048]),
        "xop0b": np.ascontiguousarray(xop[0:128, 2048:5120]),
    })
    return m


def kernel(**inputs) -> np.ndarray:
    shared = _prep_shared(inputs)
    nc = _build_v2()
    in_maps = [_prep_core(inputs, shared, core) for core in range(NCORES)]
    res = run_bass_kernel_spmd(nc, in_maps, core_ids=list(range(NCORES)))
    return np.concatenate([r["out"] for r in res.results], axis=0)


if __name__ == "__main__":
    nc = _build_v2()
    print("program built ok")
